# revision 25
# baseline (speedup 1.0000x reference)
"""Trainium2 Bass kernel for nn_Attention_59047210385633.

2-D RoPE multi-head attention (B=2, N=2305, D=768, H=12, E=64), sharded
over 8 NeuronCores: each core gets one batch and 3 heads. Host sums the
4 partial wo-projections per batch.

v3 design (vs v2):
- q/k live in fp8e4m3 (scaled x8) and scores run as DoubleRow fp8
  matmuls at 0.5 cyc/row (half the v2 PE score cost). The DR second
  half carries the q-quantization residual (q ~= q8_0 + q8_1), so the
  q-side fp8 error cancels; only the k-side error remains.
- a ones-row (q:128/0, k:64) rides the 65th contraction partition, so
  every score lands in PSUM as 64*s + 8192; the Act exp path uses
  bias=-16 scale=0.125/64, and the poly path gets its "1 +" for free.
- a subset of chunk-groups (POLY) bypasses the Act engine: exp is
  computed as (1 + t + t^2/2)^16 (t = s/128) via one DVE tensor_scalar
  (the only PSUM read), three Pool fp32 stages and two DVE fp16
  squarings. This moves ~17% of the exp row count off the Act
  bottleneck onto Pool/DVE slack.
- fp16 everywhere bf16 was (x, weights, rope tables, pt, v, unq/ab),
  halving quantization noise at identical modeled cost.
- outproj osb copies all on DVE; cc/ss DMAs moved off the Act queue.

Self-contained: hardcodes all shapes; only needs numpy + concourse.
"""

import numpy as np

import bass_rust
import concourse.bass as bass
import concourse.mybir as mybir
import concourse.tile as tile
from concourse.bass_utils import run_bass_kernel_spmd

FP32 = mybir.dt.float32
FP16 = mybir.dt.float16
FP8 = mybir.dt.float8e4
DR = mybir.MatmulPerfMode.DoubleRow
AF = mybir.ActivationFunctionType
OP = mybir.AluOpType

B, N, D, H, E = 2, 2305, 768, 12, 64
NQ = 2306        # padded query count (token 2305 is a zero pad)
NK = 2432        # padded key count: 19 full chunks of 128
KMAX = 16
BASE = 10000.0
N1 = N2 = 48
HPC = 3          # heads per core
NCH = 19         # key chunks
KW = 2336        # k-tile width: chunks 0..17 plus the 32-wide chunk-18 read
VE = 65          # per-head v block: 64 v cols + ones col
# phase-1 panels cover all NK columns; phase-2 (query) panels only NQ
PAN1 = [(0, 512), (512, 512), (1024, 512), (1536, 512), (2048, 384)]
PAN2 = [(0, 512), (512, 512), (1024, 512), (1536, 512), (2048, 258)]
# exp groups of 3 key chunks (sg pool = 2 bufs x 3 PSUM banks); key
# chunk 18 (2 real keys) is batched across all 3 heads in one exp
GROUPS = [(0, 3), (3, 3), (6, 3), (9, 3), (12, 3), (15, 3)]
# (query-panel, score-group) pairs runnable after phase-1 panel p: group g
# needs key chunks 3g..3g+2 (tokens < (3g+3)*128) and query panel qp roped
EARLY = {
    0: [(0, 0)],
    1: [(0, 1), (1, 0)],
    2: [(0, 2), (1, 1), (1, 2)],
    3: [(0, 3), (1, 3), (0, 4), (1, 4)],
    4: [(0, 5), (1, 5), (0, 18), (1, 18)],
}
# (panel, head, group-index) triples whose exp runs on the DVE/Pool poly
# pipeline instead of the Act engine; spread so no head window gets more
# than ~2 chains (the Pool runs them while otherwise idle in phase 2)
POLY = set()


def split_excess_waits(nc):
    """walrus CoreV3 codegen allows only one sync wait per engine
    instruction; move excess waits onto NoOps inserted just before."""
    engines = {
        mybir.EngineType.PE,
        mybir.EngineType.DVE,
        mybir.EngineType.Activation,
        mybir.EngineType.Pool,
        mybir.EngineType.SP,
    }
    for f in nc.m.functions:
        for b in f.blocks:
            newl = []
            changed = False
            for ins in b.instructions:
                si = ins.sync_info
                if (
                    si is not None
                    and si.on_wait is not None
                    and len(si.on_wait) > 1
                    and ins.engine in engines
                ):
                    waits = list(si.on_wait)
                    for j, w in enumerate(waits[:-1]):
                        nop = bass_rust.InstNoOp(
                            name=f"{ins.name}-wf{j}", ins=[], outs=[]
                        )
                        nop.engine = ins.engine
                        nop.sync_info = mybir.SyncInfo(on_wait=[w], on_update=[])
                        newl.append(nop)
                    ins.sync_info = mybir.SyncInfo(
                        on_wait=[waits[-1]], on_update=list(si.on_update or [])
                    )
                    changed = True
                newl.append(ins)
            if changed:
                b.instructions = newl


def _emit(nc, tc, ctx, phases=3):
    xT = nc.dram_tensor("xT", [D, NK], FP16, kind="ExternalInput").ap()
    wqkT = nc.dram_tensor("wqkT", [D, 384], FP16, kind="ExternalInput").ap()
    wvT = nc.dram_tensor("wvT", [D, 192], FP16, kind="ExternalInput").ap()
    woT = nc.dram_tensor("woT", [192, D], FP16, kind="ExternalInput").ap()
    cc = nc.dram_tensor("cc", [128, NK], FP16, kind="ExternalInput").ap()
    ss = nc.dram_tensor("ss", [128, NK], FP16, kind="ExternalInput").ap()
    onescol = nc.dram_tensor("onescol", [128, 57], FP16, kind="ExternalInput").ap()
    qkones = nc.dram_tensor("qkones", [1, 2 * NQ + KW], FP8, kind="ExternalInput").ap()
    identd = nc.dram_tensor("identd", [128, 128], FP16, kind="ExternalInput").ap()
    out = nc.dram_tensor("out", [NQ, D], FP16, kind="ExternalOutput").ap()

    const = ctx.enter_context(tc.tile_pool(name="const", bufs=1))
    xpool = ctx.enter_context(tc.tile_pool(name="xp", bufs=2))
    tcsp = ctx.enter_context(tc.tile_pool(name="tcs", bufs=2))
    tsswp = ctx.enter_context(tc.tile_pool(name="tssw", bufs=1))
    qsump = ctx.enter_context(tc.tile_pool(name="qsum", bufs=2))
    ptp = ctx.enter_context(tc.tile_pool(name="pt", bufs=6))
    ptp18 = ctx.enter_context(tc.tile_pool(name="pt18", bufs=2))
    polyp = ctx.enter_context(tc.tile_pool(name="poly", bufs=1))
    unqp = ctx.enter_context(tc.tile_pool(name="unq", bufs=2))
    recp = ctx.enter_context(tc.tile_pool(name="rec", bufs=2))
    osbp = ctx.enter_context(tc.tile_pool(name="osb", bufs=5))
    abp = ctx.enter_context(tc.tile_pool(name="ab", bufs=2))

    # PSUM: sg 2x3 banks + shared misc 2x1 banks = 8
    ps_sg = ctx.enter_context(tc.tile_pool(name="ps_sg", bufs=2, space="PSUM"))
    ps_ms = ctx.enter_context(tc.tile_pool(name="ps_ms", bufs=2, space="PSUM"))

    # ---- constants -------------------------------------------------------
    wq_sb = const.tile([128, 6, 384], FP16)
    wqr = wqkT.rearrange("(c p) m -> p c m", p=128)
    wv_sb = const.tile([128, 6, 192], FP16)
    wvr = wvT.rearrange("(c p) m -> p c m", p=128)
    cc_sb = const.tile([128, NK], FP16)
    ss_sb = const.tile([128, NK], FP16)
    wo01 = const.tile([128, D], FP16)
    wo2 = const.tile([64, D], FP16)
    ident = const.tile([128, 128], FP16)
    bias16 = const.tile([128, 1], FP32)
    nc.gpsimd.memset(bias16, -16.0)

    # fp8 q/k tiles: q [65, 2, NQ] (half0 = fp8(8q), half1 = residual),
    # k [65, 2336] (row 64 = ones-row; cols to 2336 for the chunk-18 read)
    q8t = [const.tile([65, 2, NQ], FP8, name=f"q8_{h}") for h in range(HPC)]
    k8t = [const.tile([65, KW], FP8, name=f"k8_{h}") for h in range(HPC)]
    # ones rows (q: 128|0 across halves, k: 64) land via DMA on the idle
    # Act queue; pad columns are zeroed by the rope writes themselves

    v_sb = const.tile([128, NCH * HPC * VE], FP16)
    v_sb4 = v_sb.rearrange("p (c h e) -> p c h e", c=NCH, h=HPC)
    # chunk-18 v rows replicated at partition base 32h per head, so the
    # heads-batched pt18 tile can feed PV directly
    v18 = const.tile([66, HPC, VE], FP16)
    # preload the Exp activation table while the first DMAs are in flight
    warm = const.tile([1, 8], FP32)
    nc.gpsimd.memset(warm, 0.0)
    nc.scalar.activation(warm, warm, AF.Exp, scale=1.0)
    nc.sync.dma_start(
        out=v_sb4[:, :, :, 64:65],
        in_=onescol.rearrange("p (c h) -> p c h", c=NCH),
    )

    # rope targets: mi -> [(kind, head) for g in 0..1]; kind q/k
    rope_tgt = {
        0: [("q", 0), ("q", 1)],
        1: [("q", 2), ("k", 0)],
        2: [("k", 1), ("k", 2)],
    }
    pt_tiles = {}
    pt18_tiles = {}
    ab_tiles = {}

    def scores18(pi):
        # key chunk 18 has only 2 real keys; batch all 3 heads' scores at
        # partition bases 0/32/64 and exp them in ONE activation. Rows
        # 2304..2335 include 30 zero-pad keys so the exp input is defined.
        # fp8 non-DR matmuls on half0 only (no ones-row, no residual).
        off, w = PAN2[pi]
        sg18 = ps_sg.tile([128, 512], FP32, tag="sg", name=f"sg18_{pi}")
        for h in range(HPC):
            nc.tensor.matmul(
                sg18[32 * h:32 * h + 32, :w],
                lhsT=k8t[h][0:64, 2304:2336],
                rhs=q8t[h][0:64, 0, off:off + w],
                start=True,
                stop=True,
            )
        pt18 = ptp18.tile([66, 512], FP16, tag="pt18", name=f"pt18_{pi}")
        pt18_tiles[pi] = pt18
        nc.scalar.activation(pt18[:, :w], sg18[0:66, :w], AF.Exp,
                             scale=0.125 / 64.0)

    def get_pt(pi, h):
        if (pi, h) not in pt_tiles:
            pt_tiles[(pi, h)] = ptp.tile(
                [128, NCH - 1, 512], FP16, tag="pt", name=f"pt{pi}_{h}"
            )
        return pt_tiles[(pi, h)]

    def score_mms(pi, h, c0, cnt):
        off, w = PAN2[pi]
        sg = ps_sg.tile([128, 1536], FP32, tag="sg", name=f"sg{pi}_{h}_{c0}")
        sg3 = sg.rearrange("p (c q) -> p c q", c=3)
        kbc = k8t[h].unsqueeze(1).broadcast_to([65, 2, KW])
        for j in range(cnt):
            c = c0 + j
            nc.tensor.matmul(
                sg3[:, j, :w],
                lhsT=kbc[:, :, c * 128:(c + 1) * 128],
                rhs=q8t[h][:, :, off:off + w],
                start=True,
                stop=True,
                perf_mode=DR,
            )
        return sg3

    def scores_group(pi, h, c0, cnt):
        off, w = PAN2[pi]
        pt = get_pt(pi, h)
        sg3 = score_mms(pi, h, c0, cnt)
        nc.scalar.activation(
            pt[:, c0:c0 + cnt, :w], sg3[:, 0:cnt, :w], AF.Exp,
            bias=bias16[:, :], scale=0.125 / 64.0,
        )

    def poly_group(pi, h, c0, cnt):
        # exp offloaded: (1 + t + t^2/2)^16 with t = s/128. The single
        # PSUM read (DVE tensor_scalar) makes c = sg*2^-13 = 1+t; Pool
        # runs the fp32 stages; DVE finishes with two fp16 squarings.
        off, w = PAN2[pi]
        pt = get_pt(pi, h)
        sg3 = score_mms(pi, h, c0, cnt)
        zf = polyp.tile([128, 3, 512], FP32, tag="zf")
        zf2 = polyp.tile([128, 3, 512], FP32, tag="zf2")
        zh = polyp.tile([128, 3, 512], FP16, tag="zh")
        zh2 = polyp.tile([128, 3, 512], FP16, tag="zh2")
        nc.vector.tensor_scalar(
            out=zf[:, 0:cnt, :w], in0=sg3[:, 0:cnt, :w],
            scalar1=float(2.0 ** -13), scalar2=0.0, op0=OP.mult, op1=OP.add,
        )
        nc.gpsimd.tensor_tensor(
            out=zf2[:, 0:cnt, :w], in0=zf[:, 0:cnt, :w],
            in1=zf[:, 0:cnt, :w], op=OP.mult,
        )
        nc.gpsimd.tensor_scalar(
            out=zf[:, 0:cnt, :w], in0=zf2[:, 0:cnt, :w],
            scalar1=0.5, scalar2=0.5, op0=OP.mult, op1=OP.add,
        )
        nc.gpsimd.tensor_tensor(
            out=zf2[:, 0:cnt, :w], in0=zf[:, 0:cnt, :w],
            in1=zf[:, 0:cnt, :w], op=OP.mult,
        )
        nc.gpsimd.tensor_tensor(
            out=zh[:, 0:cnt, :w], in0=zf2[:, 0:cnt, :w],
            in1=zf2[:, 0:cnt, :w], op=OP.mult,
        )
        nc.gpsimd.tensor_tensor(
            out=zh2[:, 0:cnt, :w], in0=zh[:, 0:cnt, :w],
            in1=zh[:, 0:cnt, :w], op=OP.mult,
        )
        nc.gpsimd.tensor_tensor(
            out=pt[:, c0:c0 + cnt, :w], in0=zh2[:, 0:cnt, :w],
            in1=zh2[:, 0:cnt, :w], op=OP.mult,
        )

    def scores_exp(pi, h, groups=None):
        gl = list(enumerate(groups if groups is not None else GROUPS))
        gl.sort(key=lambda t: (pi, h, t[0]) not in POLY)
        for gi, (c0, cnt) in gl:
            if (pi, h, gi) in POLY:
                poly_group(pi, h, c0, cnt)
            else:
                scores_group(pi, h, c0, cnt)

    def get_ab(pi):
        if pi not in ab_tiles:
            ab01 = abp.tile([128, 512], FP16, tag="ab01", name=f"ab01_{pi}")
            ab2 = abp.tile([64, 512], FP16, tag="ab2", name=f"ab2_{pi}")
            ab_tiles[pi] = (ab01, ab2)
        return ab_tiles[pi]

    def pv_qsub(pi, h, q0, qw, tp_pool="ms", po_pool="ms"):
        off, w = PAN2[pi]
        pt = pt_tiles[(pi, h)]
        ab01, ab2 = get_ab(pi)
        abt, ab_base = [(ab01, 0), (ab01, 64), (ab2, 0)][h]
        po = (ps_ms if po_pool == "ms" else ps_sg).tile(
            [128, 512], FP32, tag=po_pool, name=f"po{pi}_{h}_{q0}"
        )
        pt18 = pt18_tiles[pi]
        for c in range(NCH - 1):
            nc.tensor.matmul(
                po[:qw, 0:VE],
                lhsT=pt[:, c, q0:q0 + qw],
                rhs=v_sb4[:, c, h, :],
                start=(c == 0),
                stop=False,
            )
        nc.tensor.matmul(
            po[:qw, 0:VE],
            lhsT=pt18[32 * h:32 * h + 2, q0:q0 + qw],
            rhs=v18[32 * h:32 * h + 2, h, :],
            start=False,
            stop=True,
        )
        # normalize in place: unq = po[:, 0:64] * (1 / den)
        rec = recp.tile([128, 1], FP32, tag="rec")
        with nc.allow_low_precision(reason="softmax denominators are ~2e3"):
            nc.vector.reciprocal(rec[:qw, :], po[:qw, 64:65])
        unq = unqp.tile([128, 64], FP16, tag="unq")
        nc.vector.tensor_scalar_mul(unq[:qw, :], po[:qw, 0:64], rec[:qw, :])
        tp = (ps_ms if tp_pool == "ms" else ps_sg).tile(
            [128, 1024], FP16, tag=tp_pool, name=f"tp{pi}_{h}_{q0}"
        )
        nc.tensor.transpose(tp[0:64, 0:qw], unq[:qw, 0:64], ident[0:qw, 0:qw])
        nc.vector.tensor_copy(
            abt[ab_base:ab_base + 64, q0:q0 + qw], tp[0:64, 0:qw]
        )

    def pv_head(pi, h, pool="ms"):
        off, w = PAN2[pi]
        q0 = 0
        while q0 < w:
            qw = min(128, w - q0)
            pv_qsub(pi, h, q0, qw, tp_pool=pool, po_pool=pool)
            q0 += qw
        pt_tiles.pop((pi, h))

    def outproj(pi, q0, qw, pool="ms", act_osb=False):
        off, _ = PAN2[pi]
        ab01, ab2 = ab_tiles[pi]
        t0 = q0
        while t0 < q0 + qw:
            tw = min(128, q0 + qw - t0)
            for half in range(2):
                op_ps = (ps_ms if pool == "ms" else ps_sg).tile(
                    [128, 512], FP32, tag=pool, name=f"op{pi}_{t0}_{half}"
                )
                nc.tensor.matmul(
                    op_ps[:tw, 0:384],
                    lhsT=ab01[:, t0:t0 + tw],
                    rhs=wo01[:, half * 384:half * 384 + 384],
                    start=True,
                    stop=False,
                )
                nc.tensor.matmul(
                    op_ps[:tw, 0:384],
                    lhsT=ab2[0:64, t0:t0 + tw],
                    rhs=wo2[:, half * 384:half * 384 + 384],
                    start=False,
                    stop=True,
                )
                osb = osbp.tile([128, 384], FP16, tag="osb")
                if act_osb:
                    nc.scalar.copy(osb[:tw, :], op_ps[:tw, 0:384])
                else:
                    nc.vector.tensor_copy(osb[:tw, :], op_ps[:tw, 0:384])
                nc.sync.dma_start(
                    out=out[off + t0:off + t0 + tw,
                            half * 384:half * 384 + 384],
                    in_=osb[:tw, :],
                )
            t0 += tw

    # ---- phase 1: QK projection + rope; V in [tok, e]; early scores ------
    xTr = xT.rearrange("(c p) n -> p c n", p=128)
    for pi, (off, w) in enumerate(PAN1):
        xp = xpool.tile([128, 6, 512], FP16, tag="xp")
        if pi == 0:
            # startup DMAs spread across idle engine queues so the x panel,
            # weights and rope tables land in parallel
            nc.sync.dma_start(out=xp[:, :, :w], in_=xTr[:, :, off:off + w])
            nc.sync.dma_start(out=wq_sb, in_=wqr)
            nc.scalar.dma_start(out=cc_sb, in_=cc)
            nc.scalar.dma_start(out=ss_sb, in_=ss)
            nc.sync.dma_start(out=wv_sb, in_=wvr)
            for h in range(HPC):
                nc.scalar.dma_start(
                    out=k8t[h][64:65, :], in_=qkones[:, 2 * NQ:2 * NQ + KW]
                )
                nc.scalar.dma_start(
                    out=q8t[h][64:65, :, :].rearrange("p a b -> p (a b)"),
                    in_=qkones[:, 0:2 * NQ],
                )
        else:
            nc.sync.dma_start(out=xp[:, :, :w], in_=xTr[:, :, off:off + w])
        if pi == 1:
            nc.sync.dma_start(out=ident, in_=identd)
            nc.sync.dma_start(out=wo01, in_=woT[0:128, :])
            nc.sync.dma_start(out=wo2, in_=woT[128:192, :])
        # qk projection chunks + rope; mults on DVE, rest on Pool
        for mi in range(3):
            qp = ps_ms.tile([128, 512], FP32, tag="ms", name=f"qk{pi}_{mi}")
            for kc in range(6):
                nc.tensor.matmul(
                    qp[:, :w],
                    lhsT=wq_sb[:, kc, mi * 128:(mi + 1) * 128],
                    rhs=xp[:, kc, :w],
                    start=(kc == 0),
                    stop=(kc == 5),
                )
            tcs = tcsp.tile([128, 512], FP16, tag="tcs")
            nc.vector.tensor_tensor(
                out=tcs[:, :w], in0=qp[:, :w], in1=cc_sb[:, off:off + w],
                op=OP.mult,
            )
            tss = tcsp.tile([128, 512], FP16, tag="tss")
            nc.vector.tensor_tensor(
                out=tss[:, :w], in0=qp[:, :w], in1=ss_sb[:, off:off + w],
                op=OP.mult,
            )
            # DVE swap-copies tss with the sign folded in (fp16 4x mode),
            # then one Pool add per 64-block target
            tssw = tsswp.tile([128, 512], FP16, tag="tssw")
            for g in range(2):
                r = slice(g * 64, g * 64 + 32)
                i = slice(g * 64 + 32, g * 64 + 64)
                nc.vector.tensor_scalar_mul(tssw[r, :w], tss[i, :w], -1.0)
                nc.vector.tensor_copy(tssw[i, :w], tss[r, :w])
            for g in range(2):
                kind, h = rope_tgt[mi][g]
                if kind == "k":
                    kw = min(w, KW - off)
                    if kw <= 0:
                        continue
                    nc.gpsimd.tensor_tensor(
                        out=k8t[h][0:64, off:off + kw],
                        in0=tcs[g * 64:g * 64 + 64, :kw],
                        in1=tssw[g * 64:g * 64 + 64, :kw],
                        op=OP.add,
                    )
                else:
                    qw = min(w, NQ - off)
                    if qw <= 0:
                        continue
                    qsum = qsump.tile([64, 512], FP16, tag="qsum")
                    nc.gpsimd.tensor_tensor(
                        out=qsum[:, :qw],
                        in0=tcs[g * 64:g * 64 + 64, :qw],
                        in1=tssw[g * 64:g * 64 + 64, :qw],
                        op=OP.add,
                    )
                    nc.gpsimd.tensor_copy(
                        q8t[h][0:64, 0, off:off + qw], qsum[:, :qw]
                    )
                    nc.gpsimd.tensor_tensor(
                        out=q8t[h][0:64, 1, off:off + qw],
                        in0=qsum[:, :qw],
                        in1=q8t[h][0:64, 0, off:off + qw],
                        op=OP.subtract,
                    )
        # early scores for query-panels 0/1 on this panel's key chunks
        for qp_, g in EARLY[pi]:
            if g == 18:
                scores18(qp_)
            else:
                for h in range(HPC):
                    scores_group(qp_, h, *GROUPS[g])
        # v projection for this panel's key chunks, [tok, e] orientation
        for t0 in range(0, w, 128):
            ci = (off + t0) // 128
            vps = ps_ms.tile([128, 512], FP32, tag="ms", name=f"vp{ci}")
            for kc in range(6):
                nc.tensor.matmul(
                    vps[:, 0:192],
                    lhsT=xp[:, kc, t0:t0 + 128],
                    rhs=wv_sb[:, kc, :],
                    start=(kc == 0),
                    stop=(kc == 5),
                )
            nc.vector.tensor_copy(
                v_sb4[:, ci, :, 0:64],
                vps[:, 0:192].rearrange("p (h e) -> p h e", h=HPC),
            )
            if ci == NCH - 1:
                for h in range(HPC):
                    nc.gpsimd.tensor_copy(
                        v18[32 * h:32 * h + 2, h, :], v_sb4[0:2, ci, h, :]
                    )

    if phases == 1:
        return

    # ---- phase 2: attention ---------------------------------------------
    # emission order keeps Act (exp) saturated: tail work of panel p-1
    # threads between panel p's score blocks; panels 0/1 already scored
    npan = len(PAN2)
    pv_head(0, 0)
    pv_head(0, 1)
    pv_head(0, 2)
    outproj(0, 0, PAN2[0][1])
    pv_head(1, 0)
    pv_head(1, 1)
    for pi in range(2, npan):
        scores_exp(pi, 0)
        scores18(pi)
        pv_head(pi - 1, 2)
        if pi < npan - 1:
            outproj(pi - 1, 0, PAN2[pi - 1][1])
        scores_exp(pi, 1)
        pv_head(pi, 0)
        scores_exp(pi, 2)
        # the last panel's h1 chains ride the sg pool (free after the
        # final exps), keeping ms clear for outproj
        pv_head(pi, 1, pool="sg" if pi == npan - 1 else "ms")
        if pi == npan - 1:
            outproj(pi - 1, 0, PAN2[pi - 1][1], act_osb=True)
    # last panel tail: h2's chains interleave with per-qsub projections,
    # transposes ride the now-idle sg pool for extra pipeline depth
    offl, wl = PAN2[npan - 1]
    q0 = 0
    while q0 < wl:
        qw = min(128, wl - q0)
        pv_qsub(npan - 1, 2, q0, qw, tp_pool="sg", po_pool="sg")
        outproj(npan - 1, q0, qw, act_osb=True)
        q0 += qw
    pt_tiles.pop((npan - 1, 2))


_NC_CACHE = {}


def build_nc(trace_sim=False, phases=3):
    key = (bool(trace_sim), phases)
    if key in _NC_CACHE:
        return _NC_CACHE[key]
    from contextlib import ExitStack

    nc = bass.Bass("TRN2", target_bir_lowering=False, debug=False, num_devices=8)
    with tile.TileContext(nc, trace_sim=trace_sim) as tc:
        with ExitStack() as ctx:
            _emit(nc, tc, ctx, phases=phases)
    split_excess_waits(nc)
    _NC_CACHE[key] = nc
    return nc


def host_prep(x, pos0, pos1, wq, wk, wv, wo, core):
    """Per-core DRAM inputs. core -> batch b=core//4, heads 3*(core%4)+[0..2]."""
    import ml_dtypes
    fp16 = np.float16
    b = core // 4
    h0 = 3 * (core % 4)
    hs = [h0, h0 + 1, h0 + 2]

    xT = np.zeros((D, NK), np.float32)
    xT[:, :N] = x[b].T

    def perm_rows(w_h):  # evens then odds of the head dim
        return np.concatenate([w_h[0::2], w_h[1::2]], axis=0)

    wq_rows = np.concatenate([perm_rows(wq[h * E:(h + 1) * E]) for h in hs], 0)
    wk_rows = np.concatenate([perm_rows(wk[h * E:(h + 1) * E]) for h in hs], 0)
    wqkT = np.ascontiguousarray(np.concatenate([wq_rows, wk_rows], 0).T)
    wv_rows = np.concatenate([wv[h * E:(h + 1) * E] for h in hs], 0)
    wvT = np.ascontiguousarray(wv_rows.T)

    wo_cols = np.concatenate([wo[:, h * E:(h + 1) * E] for h in hs], 1)
    woT = np.ascontiguousarray(wo_cols.T)

    theta = 1.0 / (BASE ** (np.arange(KMAX, dtype=np.float32) / KMAX))
    i1, i2 = np.meshgrid(np.arange(N1), np.arange(N2), indexing="ij")
    ang0 = pos0[b][i1.ravel()][:, None] * theta[None, :]
    ang1 = pos1[b][i2.ravel()][:, None] * theta[None, :]
    ang = np.concatenate([ang0, ang1], 1).astype(np.float32)  # [N-1, 32]
    cos = np.ones((32, NK), np.float32)   # col 0 (CLS) and pad cols: identity
    sin = np.zeros((32, NK), np.float32)
    cos[:, 1:N] = np.cos(ang).T
    sin[:, 1:N] = np.sin(ang).T
    # x8 fp8 scaling folded into the rope tables
    cc = np.ascontiguousarray(np.tile(cos * 8.0, (4, 1))).astype(fp16)
    ss = np.ascontiguousarray(np.tile(sin * 8.0, (4, 1))).astype(fp16)
    onescol = np.ones((128, NCH, HPC), np.float32)
    onescol[1:, NCH - 1, :] = 0.0  # pad keys contribute nothing
    identd = np.eye(128, dtype=np.float32)
    qkones = np.zeros((1, 2 * NQ + KW), np.float32)
    qkones[0, :NQ] = 128.0
    qkones[0, 2 * NQ:] = 64.0
    return {
        "xT": xT.astype(fp16),
        "wqkT": wqkT.astype(fp16),
        "wvT": wvT.astype(fp16),
        "woT": woT.astype(fp16),
        "cc": cc, "ss": ss,
        "onescol": np.ascontiguousarray(
            onescol.reshape(128, NCH * HPC)).astype(fp16),
        "identd": identd.astype(fp16),
        "qkones": qkones.astype(ml_dtypes.float8_e4m3),
    }


def kernel(x, pos0, pos1, wq, wk, wv, wo):
    x = np.asarray(x, np.float32)
    pos0 = np.asarray(pos0, np.float32)
    pos1 = np.asarray(pos1, np.float32)
    wq = np.asarray(wq, np.float32)
    wk = np.asarray(wk, np.float32)
    wv = np.asarray(wv, np.float32)
    wo = np.asarray(wo, np.float32)

    in_maps = [
        host_prep(x, pos0, pos1, wq, wk, wv, wo, core) for core in range(8)
    ]
    nc = build_nc()
    res = run_bass_kernel_spmd(nc, in_maps, core_ids=list(range(8)))
    out = np.zeros((B, N, D), np.float32)
    for core in range(8):
        out[core // 4] += np.asarray(res.results[core]["out"][:N], np.float32)
    return out


# revision 30
# speedup vs baseline: 1.0207x; 1.0207x over previous
"""Trainium2 Bass kernel for nn_Attention_59047210385633.

2-D RoPE multi-head attention (B=2, N=2305, D=768, H=12, E=64), sharded
over 8 NeuronCores: each core gets one batch and 3 heads. Host sums the
4 partial wo-projections per batch.

v3 design (vs v2):
- q/k live in fp8e4m3 (scaled x8) and scores run as DoubleRow fp8
  matmuls at 0.5 cyc/row (half the v2 PE score cost). The DR second
  half carries the q-quantization residual (q ~= q8_0 + q8_1), so the
  q-side fp8 error cancels; only the k-side error remains.
- a ones-row (q:128/0, k:64) rides the 65th contraction partition, so
  every score lands in PSUM as 64*s + 8192; the Act exp path uses
  bias=-16 scale=0.125/64, and the poly path gets its "1 +" for free.
- a subset of chunk-groups (POLY) bypasses the Act engine: exp is
  computed as (1 + t + t^2/2)^16 (t = s/128) via one DVE tensor_scalar
  (the only PSUM read), three Pool fp32 stages and two DVE fp16
  squarings. This moves ~17% of the exp row count off the Act
  bottleneck onto Pool/DVE slack.
- fp16 everywhere bf16 was (x, weights, rope tables, pt, v, unq/ab),
  halving quantization noise at identical modeled cost.
- outproj osb copies all on DVE; cc/ss DMAs moved off the Act queue.

Self-contained: hardcodes all shapes; only needs numpy + concourse.
"""

import numpy as np

import bass_rust
import concourse.bass as bass
import concourse.mybir as mybir
import concourse.tile as tile
from concourse.bass_utils import run_bass_kernel_spmd

FP32 = mybir.dt.float32
FP16 = mybir.dt.float16
FP8 = mybir.dt.float8e4
DR = mybir.MatmulPerfMode.DoubleRow
AF = mybir.ActivationFunctionType
OP = mybir.AluOpType

B, N, D, H, E = 2, 2305, 768, 12, 64
NQ = 2306        # padded query count (token 2305 is a zero pad)
NK = 2432        # padded key count: 19 full chunks of 128
KMAX = 16
BASE = 10000.0
N1 = N2 = 48
HPC = 3          # heads per core
NCH = 19         # key chunks
KW = 2336        # k-tile width: chunks 0..17 plus the 32-wide chunk-18 read
VE = 65          # per-head v block: 64 v cols + ones col
# phase-1 panels cover all NK columns; phase-2 (query) panels only NQ
PAN1 = [(0, 512), (512, 512), (1024, 512), (1536, 512), (2048, 384)]
PAN2 = [(0, 512), (512, 512), (1024, 512), (1536, 512), (2048, 258)]
# exp groups of 3 key chunks (sg pool = 2 bufs x 3 PSUM banks); key
# chunk 18 (2 real keys) is batched across all 3 heads in one exp
GROUPS = [(0, 3), (3, 3), (6, 3), (9, 3), (12, 3), (15, 3)]
# (query-panel, score-group) pairs runnable after phase-1 panel p: group g
# needs key chunks 3g..3g+2 (tokens < (3g+3)*128) and query panel qp roped
EARLY = {
    0: [(0, 0)],
    1: [(0, 1), (1, 0)],
    2: [(0, 2), (1, 1), (1, 2)],
    3: [(0, 3), (1, 3), (0, 4), (1, 4)],
    4: [(0, 5), (1, 5), (0, 18), (1, 18)],
}
# (panel, head, group-index) triples whose exp runs on the DVE/Pool poly
# pipeline instead of the Act engine; spread so no head window gets more
# than ~2 chains (the Pool runs them while otherwise idle in phase 2)
POLY = set()


def split_excess_waits(nc):
    """walrus CoreV3 codegen allows only one sync wait per engine
    instruction; move excess waits onto NoOps inserted just before."""
    engines = {
        mybir.EngineType.PE,
        mybir.EngineType.DVE,
        mybir.EngineType.Activation,
        mybir.EngineType.Pool,
        mybir.EngineType.SP,
    }
    for f in nc.m.functions:
        for b in f.blocks:
            newl = []
            changed = False
            for ins in b.instructions:
                si = ins.sync_info
                if (
                    si is not None
                    and si.on_wait is not None
                    and len(si.on_wait) > 1
                    and ins.engine in engines
                ):
                    waits = list(si.on_wait)
                    for j, w in enumerate(waits[:-1]):
                        nop = bass_rust.InstNoOp(
                            name=f"{ins.name}-wf{j}", ins=[], outs=[]
                        )
                        nop.engine = ins.engine
                        nop.sync_info = mybir.SyncInfo(on_wait=[w], on_update=[])
                        newl.append(nop)
                    ins.sync_info = mybir.SyncInfo(
                        on_wait=[waits[-1]], on_update=list(si.on_update or [])
                    )
                    changed = True
                newl.append(ins)
            if changed:
                b.instructions = newl


def _emit(nc, tc, ctx, phases=3):
    xT = nc.dram_tensor("xT", [D, NK], FP16, kind="ExternalInput").ap()
    wqkT = nc.dram_tensor("wqkT", [D, 384], FP16, kind="ExternalInput").ap()
    wvT = nc.dram_tensor("wvT", [D, 192], FP16, kind="ExternalInput").ap()
    woT = nc.dram_tensor("woT", [192, D], FP16, kind="ExternalInput").ap()
    cc = nc.dram_tensor("cc", [128, NK], FP16, kind="ExternalInput").ap()
    ss = nc.dram_tensor("ss", [128, NK], FP16, kind="ExternalInput").ap()
    onescol = nc.dram_tensor("onescol", [128, 57], FP16, kind="ExternalInput").ap()
    identd = nc.dram_tensor("identd", [128, 128], FP16, kind="ExternalInput").ap()
    out = nc.dram_tensor("out", [NQ, D], FP16, kind="ExternalOutput").ap()

    const = ctx.enter_context(tc.tile_pool(name="const", bufs=1))
    xpool = ctx.enter_context(tc.tile_pool(name="xp", bufs=2))
    tcsp = ctx.enter_context(tc.tile_pool(name="tcs", bufs=2))
    tsswp = ctx.enter_context(tc.tile_pool(name="tssw", bufs=1))
    qsump = ctx.enter_context(tc.tile_pool(name="qsum", bufs=2))
    ptp = ctx.enter_context(tc.tile_pool(name="pt", bufs=6))
    ptp18 = ctx.enter_context(tc.tile_pool(name="pt18", bufs=2))
    polyp = ctx.enter_context(tc.tile_pool(name="poly", bufs=1))
    unqp = ctx.enter_context(tc.tile_pool(name="unq", bufs=2))
    recp = ctx.enter_context(tc.tile_pool(name="rec", bufs=2))
    osbp = ctx.enter_context(tc.tile_pool(name="osb", bufs=5))
    abp = ctx.enter_context(tc.tile_pool(name="ab", bufs=2))

    # PSUM: sg 2x3 banks + shared misc 2x1 banks = 8
    ps_sg = ctx.enter_context(tc.tile_pool(name="ps_sg", bufs=2, space="PSUM"))
    ps_ms = ctx.enter_context(tc.tile_pool(name="ps_ms", bufs=2, space="PSUM"))

    # ---- constants -------------------------------------------------------
    wq_sb = const.tile([128, 6, 384], FP16)
    wqr = wqkT.rearrange("(c p) m -> p c m", p=128)
    wv_sb = const.tile([128, 6, 192], FP16)
    wvr = wvT.rearrange("(c p) m -> p c m", p=128)
    cc_sb = const.tile([128, NK], FP16)
    ss_sb = const.tile([128, NK], FP16)
    wo01 = const.tile([128, D], FP16)
    wo2 = const.tile([64, D], FP16)
    ident = const.tile([128, 128], FP16)

    # fp8 q/k tiles: q [64, 2, NQ] (half0 = fp8(8q), half1 = residual),
    # k [64, 2336] (cols to 2336 for the chunk-18 read); pad columns are
    # zeroed by the rope writes themselves
    q8t = [const.tile([64, 2, NQ], FP8, name=f"q8_{h}") for h in range(HPC)]
    k8t = [const.tile([64, KW], FP8, name=f"k8_{h}") for h in range(HPC)]

    v_sb = const.tile([128, NCH * HPC * VE], FP16)
    v_sb4 = v_sb.rearrange("p (c h e) -> p c h e", c=NCH, h=HPC)
    # chunk-18 v rows replicated at partition base 32h per head, so the
    # heads-batched pt18 tile can feed PV directly
    v18 = const.tile([66, HPC, VE], FP16)
    # preload the Exp activation table while the first DMAs are in flight
    warm = const.tile([1, 8], FP32)
    nc.gpsimd.memset(warm, 0.0)
    nc.scalar.activation(warm, warm, AF.Exp, scale=1.0)

    # rope targets: mi -> [(kind, head) for g in 0..1]; kind q/k
    rope_tgt = {
        0: [("q", 0), ("q", 1)],
        1: [("q", 2), ("k", 0)],
        2: [("k", 1), ("k", 2)],
    }
    pt_tiles = {}
    pt18_tiles = {}
    ab_tiles = {}

    def scores18(pi):
        # key chunk 18 has only 2 real keys; batch all 3 heads' scores at
        # partition bases 0/32/64 and exp them in ONE activation. Rows
        # 2304..2335 include 30 zero-pad keys so the exp input is defined.
        # fp8 non-DR matmuls on half0 only (no ones-row, no residual).
        off, w = PAN2[pi]
        sg18 = ps_sg.tile([128, 512], FP32, tag="sg", name=f"sg18_{pi}")
        for h in range(HPC):
            nc.tensor.matmul(
                sg18[32 * h:32 * h + 32, :w],
                lhsT=k8t[h][0:64, 2304:2336],
                rhs=q8t[h][0:64, 0, off:off + w],
                start=True,
                stop=True,
            )
        pt18 = ptp18.tile([66, 512], FP16, tag="pt18", name=f"pt18_{pi}")
        pt18_tiles[pi] = pt18
        nc.scalar.activation(pt18[:, :w], sg18[0:66, :w], AF.Exp,
                             scale=0.125 / 64.0)

    def get_pt(pi, h):
        if (pi, h) not in pt_tiles:
            pt_tiles[(pi, h)] = ptp.tile(
                [128, NCH - 1, 512], FP16, tag="pt", name=f"pt{pi}_{h}"
            )
        return pt_tiles[(pi, h)]

    def score_mms(pi, h, c0, cnt):
        off, w = PAN2[pi]
        sg = ps_sg.tile([128, 1536], FP32, tag="sg", name=f"sg{pi}_{h}_{c0}")
        sg3 = sg.rearrange("p (c q) -> p c q", c=3)
        kbc = k8t[h].unsqueeze(1).broadcast_to([64, 2, KW])
        for j in range(cnt):
            c = c0 + j
            nc.tensor.matmul(
                sg3[:, j, :w],
                lhsT=kbc[:, :, c * 128:(c + 1) * 128],
                rhs=q8t[h][:, :, off:off + w],
                start=True,
                stop=True,
                perf_mode=DR,
            )
        return sg3

    def scores_group(pi, h, c0, cnt):
        off, w = PAN2[pi]
        pt = get_pt(pi, h)
        sg3 = score_mms(pi, h, c0, cnt)
        nc.scalar.activation(
            pt[:, c0:c0 + cnt, :w], sg3[:, 0:cnt, :w], AF.Exp,
            scale=0.125 / 64.0,
        )

    def poly_group(pi, h, c0, cnt):
        # exp offloaded: (1 + t + t^2/2)^16 with t = s/128. The single
        # PSUM read (DVE tensor_scalar) makes c = sg*2^-13 = 1+t; Pool
        # runs the fp32 stages; DVE finishes with two fp16 squarings.
        off, w = PAN2[pi]
        pt = get_pt(pi, h)
        sg3 = score_mms(pi, h, c0, cnt)
        zf = polyp.tile([128, 3, 512], FP32, tag="zf")
        zf2 = polyp.tile([128, 3, 512], FP32, tag="zf2")
        zh = polyp.tile([128, 3, 512], FP16, tag="zh")
        zh2 = polyp.tile([128, 3, 512], FP16, tag="zh2")
        nc.vector.tensor_scalar(
            out=zf[:, 0:cnt, :w], in0=sg3[:, 0:cnt, :w],
            scalar1=float(2.0 ** -13), scalar2=0.0, op0=OP.mult, op1=OP.add,
        )
        nc.gpsimd.tensor_tensor(
            out=zf2[:, 0:cnt, :w], in0=zf[:, 0:cnt, :w],
            in1=zf[:, 0:cnt, :w], op=OP.mult,
        )
        nc.gpsimd.tensor_scalar(
            out=zf[:, 0:cnt, :w], in0=zf2[:, 0:cnt, :w],
            scalar1=0.5, scalar2=0.5, op0=OP.mult, op1=OP.add,
        )
        nc.gpsimd.tensor_tensor(
            out=zf2[:, 0:cnt, :w], in0=zf[:, 0:cnt, :w],
            in1=zf[:, 0:cnt, :w], op=OP.mult,
        )
        nc.gpsimd.tensor_tensor(
            out=zh[:, 0:cnt, :w], in0=zf2[:, 0:cnt, :w],
            in1=zf2[:, 0:cnt, :w], op=OP.mult,
        )
        nc.gpsimd.tensor_tensor(
            out=zh2[:, 0:cnt, :w], in0=zh[:, 0:cnt, :w],
            in1=zh[:, 0:cnt, :w], op=OP.mult,
        )
        nc.gpsimd.tensor_tensor(
            out=pt[:, c0:c0 + cnt, :w], in0=zh2[:, 0:cnt, :w],
            in1=zh2[:, 0:cnt, :w], op=OP.mult,
        )

    def scores_exp(pi, h, groups=None):
        gl = list(enumerate(groups if groups is not None else GROUPS))
        gl.sort(key=lambda t: (pi, h, t[0]) not in POLY)
        for gi, (c0, cnt) in gl:
            if (pi, h, gi) in POLY:
                poly_group(pi, h, c0, cnt)
            else:
                scores_group(pi, h, c0, cnt)

    def get_ab(pi):
        if pi not in ab_tiles:
            ab01 = abp.tile([128, 512], FP16, tag="ab01", name=f"ab01_{pi}")
            ab2 = abp.tile([64, 512], FP16, tag="ab2", name=f"ab2_{pi}")
            ab_tiles[pi] = (ab01, ab2)
        return ab_tiles[pi]

    def pv_qsub(pi, h, q0, qw, tp_pool="ms", po_pool="ms"):
        off, w = PAN2[pi]
        pt = pt_tiles[(pi, h)]
        ab01, ab2 = get_ab(pi)
        abt, ab_base = [(ab01, 0), (ab01, 64), (ab2, 0)][h]
        po = (ps_ms if po_pool == "ms" else ps_sg).tile(
            [128, 512], FP32, tag=po_pool, name=f"po{pi}_{h}_{q0}"
        )
        pt18 = pt18_tiles[pi]
        for c in range(NCH - 1):
            nc.tensor.matmul(
                po[:qw, 0:VE],
                lhsT=pt[:, c, q0:q0 + qw],
                rhs=v_sb4[:, c, h, :],
                start=(c == 0),
                stop=False,
            )
        nc.tensor.matmul(
            po[:qw, 0:VE],
            lhsT=pt18[32 * h:32 * h + 2, q0:q0 + qw],
            rhs=v18[32 * h:32 * h + 2, h, :],
            start=False,
            stop=True,
        )
        # normalize in place: unq = po[:, 0:64] * (1 / den)
        rec = recp.tile([128, 1], FP32, tag="rec")
        with nc.allow_low_precision(reason="softmax denominators are ~2e3"):
            nc.vector.reciprocal(rec[:qw, :], po[:qw, 64:65])
        unq = unqp.tile([128, 64], FP16, tag="unq")
        nc.vector.tensor_scalar_mul(unq[:qw, :], po[:qw, 0:64], rec[:qw, :])
        tp = (ps_ms if tp_pool == "ms" else ps_sg).tile(
            [128, 1024], FP16, tag=tp_pool, name=f"tp{pi}_{h}_{q0}"
        )
        nc.tensor.transpose(tp[0:64, 0:qw], unq[:qw, 0:64], ident[0:qw, 0:qw])
        nc.vector.tensor_copy(
            abt[ab_base:ab_base + 64, q0:q0 + qw], tp[0:64, 0:qw]
        )

    def pv_head(pi, h, pool="ms"):
        off, w = PAN2[pi]
        q0 = 0
        while q0 < w:
            qw = min(128, w - q0)
            pv_qsub(pi, h, q0, qw, tp_pool=pool, po_pool=pool)
            q0 += qw
        pt_tiles.pop((pi, h))

    def outproj(pi, q0, qw, pool="ms", act_osb=False):
        off, _ = PAN2[pi]
        ab01, ab2 = ab_tiles[pi]
        t0 = q0
        while t0 < q0 + qw:
            tw = min(128, q0 + qw - t0)
            for half in range(2):
                op_ps = (ps_ms if pool == "ms" else ps_sg).tile(
                    [128, 512], FP32, tag=pool, name=f"op{pi}_{t0}_{half}"
                )
                nc.tensor.matmul(
                    op_ps[:tw, 0:384],
                    lhsT=ab01[:, t0:t0 + tw],
                    rhs=wo01[:, half * 384:half * 384 + 384],
                    start=True,
                    stop=False,
                )
                nc.tensor.matmul(
                    op_ps[:tw, 0:384],
                    lhsT=ab2[0:64, t0:t0 + tw],
                    rhs=wo2[:, half * 384:half * 384 + 384],
                    start=False,
                    stop=True,
                )
                osb = osbp.tile([128, 384], FP16, tag="osb")
                if act_osb:
                    nc.scalar.copy(osb[:tw, :], op_ps[:tw, 0:384])
                else:
                    nc.vector.tensor_copy(osb[:tw, :], op_ps[:tw, 0:384])
                nc.sync.dma_start(
                    out=out[off + t0:off + t0 + tw,
                            half * 384:half * 384 + 384],
                    in_=osb[:tw, :],
                )
            t0 += tw

    # ---- phase 1: QK projection + rope; V in [tok, e]; early scores ------
    xTr = xT.rearrange("(c p) n -> p c n", p=128)
    for pi, (off, w) in enumerate(PAN1):
        xp = xpool.tile([128, 6, 512], FP16, tag="xp")
        if pi == 0:
            # startup DMAs spread across idle engine queues so the x panel,
            # weights and rope tables land in parallel
            nc.sync.dma_start(out=xp[:, :, :w], in_=xTr[:, :, off:off + w])
            nc.scalar.dma_start(out=wq_sb, in_=wqr)
            nc.scalar.dma_start(out=cc_sb, in_=cc)
            nc.scalar.dma_start(out=ss_sb, in_=ss)
            nc.sync.dma_start(out=wv_sb, in_=wvr)
            # PE p-state warm-up: dummy full-shape matmuls keep the PE
            # continuously busy from ~t=1us so the real projections run
            # at full clock; outputs are never read
            pew = const.tile([128, 512], FP16, name="pew")
            nc.gpsimd.memset(pew, 0.0)
            for wi in range(6):
                wps = ps_ms.tile([128, 512], FP32, tag="ms", name=f"pew{wi}")
                nc.tensor.matmul(
                    wps[:, :], lhsT=pew[:, 0:128], rhs=pew[:, :],
                    start=True, stop=True,
                )
        else:
            nc.sync.dma_start(out=xp[:, :, :w], in_=xTr[:, :, off:off + w])
        if pi == 1:
            nc.sync.dma_start(
                out=v_sb4[:, :, :, 64:65],
                in_=onescol.rearrange("p (c h) -> p c h", c=NCH),
            )
            nc.sync.dma_start(out=ident, in_=identd)
            nc.sync.dma_start(out=wo01, in_=woT[0:128, :])
            nc.sync.dma_start(out=wo2, in_=woT[128:192, :])
        # qk projection chunks + rope; mults on DVE, rest on Pool
        for mi in range(3):
            qp = ps_ms.tile([128, 512], FP32, tag="ms", name=f"qk{pi}_{mi}")
            for kc in range(6):
                nc.tensor.matmul(
                    qp[:, :w],
                    lhsT=wq_sb[:, kc, mi * 128:(mi + 1) * 128],
                    rhs=xp[:, kc, :w],
                    start=(kc == 0),
                    stop=(kc == 5),
                )
            tcs = tcsp.tile([128, 512], FP16, tag="tcs")
            nc.vector.tensor_tensor(
                out=tcs[:, :w], in0=qp[:, :w], in1=cc_sb[:, off:off + w],
                op=OP.mult,
            )
            tss = tcsp.tile([128, 512], FP16, tag="tss")
            nc.vector.tensor_tensor(
                out=tss[:, :w], in0=qp[:, :w], in1=ss_sb[:, off:off + w],
                op=OP.mult,
            )
            # DVE swap-copies tss with the sign folded in (fp16 4x mode),
            # then one Pool add per 64-block target
            tssw = tsswp.tile([128, 512], FP16, tag="tssw")
            for g in range(2):
                r = slice(g * 64, g * 64 + 32)
                i = slice(g * 64 + 32, g * 64 + 64)
                nc.vector.tensor_scalar_mul(tssw[r, :w], tss[i, :w], -1.0)
                nc.vector.tensor_copy(tssw[i, :w], tss[r, :w])
            for g in range(2):
                kind, h = rope_tgt[mi][g]
                if kind == "k":
                    kw = min(w, KW - off)
                    if kw <= 0:
                        continue
                    nc.gpsimd.tensor_tensor(
                        out=k8t[h][0:64, off:off + kw],
                        in0=tcs[g * 64:g * 64 + 64, :kw],
                        in1=tssw[g * 64:g * 64 + 64, :kw],
                        op=OP.add,
                    )
                else:
                    qw = min(w, NQ - off)
                    if qw <= 0:
                        continue
                    qsum = qsump.tile([64, 512], FP16, tag="qsum")
                    nc.gpsimd.tensor_tensor(
                        out=qsum[:, :qw],
                        in0=tcs[g * 64:g * 64 + 64, :qw],
                        in1=tssw[g * 64:g * 64 + 64, :qw],
                        op=OP.add,
                    )
                    nc.gpsimd.tensor_copy(
                        q8t[h][0:64, 0, off:off + qw], qsum[:, :qw]
                    )
                    nc.gpsimd.tensor_tensor(
                        out=q8t[h][0:64, 1, off:off + qw],
                        in0=qsum[:, :qw],
                        in1=q8t[h][0:64, 0, off:off + qw],
                        op=OP.subtract,
                    )
        # early scores for query-panels 0/1 on this panel's key chunks
        for qp_, g in EARLY[pi]:
            if g == 18:
                scores18(qp_)
            else:
                for h in range(HPC):
                    scores_group(qp_, h, *GROUPS[g])
        # v projection for this panel's key chunks, [tok, e] orientation
        for t0 in range(0, w, 128):
            ci = (off + t0) // 128
            vps = ps_ms.tile([128, 512], FP32, tag="ms", name=f"vp{ci}")
            for kc in range(6):
                nc.tensor.matmul(
                    vps[:, 0:192],
                    lhsT=xp[:, kc, t0:t0 + 128],
                    rhs=wv_sb[:, kc, :],
                    start=(kc == 0),
                    stop=(kc == 5),
                )
            nc.vector.tensor_copy(
                v_sb4[:, ci, :, 0:64],
                vps[:, 0:192].rearrange("p (h e) -> p h e", h=HPC),
            )
            if ci == NCH - 1:
                for h in range(HPC):
                    nc.gpsimd.tensor_copy(
                        v18[32 * h:32 * h + 2, h, :], v_sb4[0:2, ci, h, :]
                    )

    if phases == 1:
        return

    # ---- phase 2: attention ---------------------------------------------
    # emission order keeps Act (exp) saturated: tail work of panel p-1
    # threads between panel p's score blocks; panels 0/1 already scored
    npan = len(PAN2)
    pv_head(0, 0)
    pv_head(0, 1)
    pv_head(0, 2)
    outproj(0, 0, PAN2[0][1])
    pv_head(1, 0)
    pv_head(1, 1)
    for pi in range(2, npan):
        scores_exp(pi, 0)
        scores18(pi)
        pv_head(pi - 1, 2)
        if pi < npan - 1:
            outproj(pi - 1, 0, PAN2[pi - 1][1])
        scores_exp(pi, 1)
        pv_head(pi, 0)
        scores_exp(pi, 2)
        # the last panel's h1 chains ride the sg pool (free after the
        # final exps), keeping ms clear for outproj
        pv_head(pi, 1, pool="sg" if pi == npan - 1 else "ms")
        if pi == npan - 1:
            outproj(pi - 1, 0, PAN2[pi - 1][1])
    # last panel tail: h2's chains interleave with per-qsub projections,
    # transposes ride the now-idle sg pool for extra pipeline depth
    offl, wl = PAN2[npan - 1]
    q0 = 0
    while q0 < wl:
        qw = min(128, wl - q0)
        pv_qsub(npan - 1, 2, q0, qw, tp_pool="sg", po_pool="sg")
        outproj(npan - 1, q0, qw, act_osb=True)
        q0 += qw
    pt_tiles.pop((npan - 1, 2))


_NC_CACHE = {}


def build_nc(trace_sim=False, phases=3):
    key = (bool(trace_sim), phases)
    if key in _NC_CACHE:
        return _NC_CACHE[key]
    from contextlib import ExitStack

    nc = bass.Bass("TRN2", target_bir_lowering=False, debug=False, num_devices=8)
    with tile.TileContext(nc, trace_sim=trace_sim) as tc:
        with ExitStack() as ctx:
            _emit(nc, tc, ctx, phases=phases)
    split_excess_waits(nc)
    _NC_CACHE[key] = nc
    return nc


def host_prep(x, pos0, pos1, wq, wk, wv, wo, core):
    """Per-core DRAM inputs. core -> batch b=core//4, heads 3*(core%4)+[0..2]."""
    import ml_dtypes
    fp16 = np.float16
    b = core // 4
    h0 = 3 * (core % 4)
    hs = [h0, h0 + 1, h0 + 2]

    xT = np.zeros((D, NK), np.float32)
    xT[:, :N] = x[b].T

    def perm_rows(w_h):  # evens then odds of the head dim
        return np.concatenate([w_h[0::2], w_h[1::2]], axis=0)

    wq_rows = np.concatenate([perm_rows(wq[h * E:(h + 1) * E]) for h in hs], 0)
    wk_rows = np.concatenate([perm_rows(wk[h * E:(h + 1) * E]) for h in hs], 0)
    wqkT = np.ascontiguousarray(np.concatenate([wq_rows, wk_rows], 0).T)
    wv_rows = np.concatenate([wv[h * E:(h + 1) * E] for h in hs], 0)
    wvT = np.ascontiguousarray(wv_rows.T)

    wo_cols = np.concatenate([wo[:, h * E:(h + 1) * E] for h in hs], 1)
    woT = np.ascontiguousarray(wo_cols.T)

    theta = 1.0 / (BASE ** (np.arange(KMAX, dtype=np.float32) / KMAX))
    i1, i2 = np.meshgrid(np.arange(N1), np.arange(N2), indexing="ij")
    ang0 = pos0[b][i1.ravel()][:, None] * theta[None, :]
    ang1 = pos1[b][i2.ravel()][:, None] * theta[None, :]
    ang = np.concatenate([ang0, ang1], 1).astype(np.float32)  # [N-1, 32]
    cos = np.ones((32, NK), np.float32)   # col 0 (CLS) and pad cols: identity
    sin = np.zeros((32, NK), np.float32)
    cos[:, 1:N] = np.cos(ang).T
    sin[:, 1:N] = np.sin(ang).T
    # x8 fp8 scaling folded into the rope tables
    cc = np.ascontiguousarray(np.tile(cos * 8.0, (4, 1))).astype(fp16)
    ss = np.ascontiguousarray(np.tile(sin * 8.0, (4, 1))).astype(fp16)
    onescol = np.ones((128, NCH, HPC), np.float32)
    onescol[1:, NCH - 1, :] = 0.0  # pad keys contribute nothing
    identd = np.eye(128, dtype=np.float32)
    return {
        "xT": xT.astype(fp16),
        "wqkT": wqkT.astype(fp16),
        "wvT": wvT.astype(fp16),
        "woT": woT.astype(fp16),
        "cc": cc, "ss": ss,
        "onescol": np.ascontiguousarray(
            onescol.reshape(128, NCH * HPC)).astype(fp16),
        "identd": identd.astype(fp16),
    }


def kernel(x, pos0, pos1, wq, wk, wv, wo):
    x = np.asarray(x, np.float32)
    pos0 = np.asarray(pos0, np.float32)
    pos1 = np.asarray(pos1, np.float32)
    wq = np.asarray(wq, np.float32)
    wk = np.asarray(wk, np.float32)
    wv = np.asarray(wv, np.float32)
    wo = np.asarray(wo, np.float32)

    in_maps = [
        host_prep(x, pos0, pos1, wq, wk, wv, wo, core) for core in range(8)
    ]
    nc = build_nc()
    res = run_bass_kernel_spmd(nc, in_maps, core_ids=list(range(8)))
    out = np.zeros((B, N, D), np.float32)
    for core in range(8):
        out[core // 4] += np.asarray(res.results[core]["out"][:N], np.float32)
    return out


# revision 31
# speedup vs baseline: 1.0344x; 1.0135x over previous
"""Trainium2 Bass kernel for nn_Attention_59047210385633.

2-D RoPE multi-head attention (B=2, N=2305, D=768, H=12, E=64), sharded
over 8 NeuronCores: each core gets one batch and 3 heads. Host sums the
4 partial wo-projections per batch.

v3 design (vs v2):
- q/k live in fp8e4m3 (scaled x8) and scores run as DoubleRow fp8
  matmuls at 0.5 cyc/row (half the v2 PE score cost). The DR second
  half carries the q-quantization residual (q ~= q8_0 + q8_1), so the
  q-side fp8 error cancels; only the k-side error remains.
- a ones-row (q:128/0, k:64) rides the 65th contraction partition, so
  every score lands in PSUM as 64*s + 8192; the Act exp path uses
  bias=-16 scale=0.125/64, and the poly path gets its "1 +" for free.
- a subset of chunk-groups (POLY) bypasses the Act engine: exp is
  computed as (1 + t + t^2/2)^16 (t = s/128) via one DVE tensor_scalar
  (the only PSUM read), three Pool fp32 stages and two DVE fp16
  squarings. This moves ~17% of the exp row count off the Act
  bottleneck onto Pool/DVE slack.
- fp16 everywhere bf16 was (x, weights, rope tables, pt, v, unq/ab),
  halving quantization noise at identical modeled cost.
- outproj osb copies all on DVE; cc/ss DMAs moved off the Act queue.

Self-contained: hardcodes all shapes; only needs numpy + concourse.
"""

import numpy as np

import bass_rust
import concourse.bass as bass
import concourse.mybir as mybir
import concourse.tile as tile
from concourse.bass_utils import run_bass_kernel_spmd

FP32 = mybir.dt.float32
FP16 = mybir.dt.float16
FP8 = mybir.dt.float8e4
DR = mybir.MatmulPerfMode.DoubleRow
AF = mybir.ActivationFunctionType
OP = mybir.AluOpType

B, N, D, H, E = 2, 2305, 768, 12, 64
NQ = 2306        # padded query count (token 2305 is a zero pad)
NK = 2432        # padded key count: 19 full chunks of 128
KMAX = 16
BASE = 10000.0
N1 = N2 = 48
HPC = 3          # heads per core
NCH = 19         # key chunks
KW = 2336        # k-tile width: chunks 0..17 plus the 32-wide chunk-18 read
VE = 65          # per-head v block: 64 v cols + ones col
# phase-1 panels cover all NK columns; phase-2 (query) panels only NQ
PAN1 = [(0, 512), (512, 512), (1024, 512), (1536, 512), (2048, 384)]
PAN2 = [(0, 512), (512, 512), (1024, 512), (1536, 512), (2048, 258)]
# exp groups of 3 key chunks (sg pool = 2 bufs x 3 PSUM banks); key
# chunk 18 (2 real keys) is batched across all 3 heads in one exp
GROUPS = [(0, 3), (3, 3), (6, 3), (9, 3), (12, 3), (15, 3)]
# (query-panel, score-group) pairs runnable after phase-1 panel p: group g
# needs key chunks 3g..3g+2 (tokens < (3g+3)*128) and query panel qp roped
EARLY = {
    0: [(0, 0)],
    1: [(0, 1), (1, 0)],
    2: [(0, 2), (1, 1), (1, 2)],
    3: [(0, 3), (1, 3), (0, 4), (1, 4)],
    4: [(0, 5), (1, 5), (0, 18), (1, 18)],
}
# (panel, head, group-index) triples whose exp runs on the DVE/Pool poly
# pipeline instead of the Act engine; spread so no head window gets more
# than ~2 chains (the Pool runs them while otherwise idle in phase 2)
POLY = set()


def split_excess_waits(nc):
    """walrus CoreV3 codegen allows only one sync wait per engine
    instruction; move excess waits onto NoOps inserted just before."""
    engines = {
        mybir.EngineType.PE,
        mybir.EngineType.DVE,
        mybir.EngineType.Activation,
        mybir.EngineType.Pool,
        mybir.EngineType.SP,
    }
    for f in nc.m.functions:
        for b in f.blocks:
            newl = []
            changed = False
            for ins in b.instructions:
                si = ins.sync_info
                if (
                    si is not None
                    and si.on_wait is not None
                    and len(si.on_wait) > 1
                    and ins.engine in engines
                ):
                    waits = list(si.on_wait)
                    for j, w in enumerate(waits[:-1]):
                        nop = bass_rust.InstNoOp(
                            name=f"{ins.name}-wf{j}", ins=[], outs=[]
                        )
                        nop.engine = ins.engine
                        nop.sync_info = mybir.SyncInfo(on_wait=[w], on_update=[])
                        newl.append(nop)
                    ins.sync_info = mybir.SyncInfo(
                        on_wait=[waits[-1]], on_update=list(si.on_update or [])
                    )
                    changed = True
                newl.append(ins)
            if changed:
                b.instructions = newl


def _emit(nc, tc, ctx, phases=3):
    xT = nc.dram_tensor("xT", [D, NK], FP16, kind="ExternalInput").ap()
    wqkT = nc.dram_tensor("wqkT", [D, 384], FP16, kind="ExternalInput").ap()
    wvT = nc.dram_tensor("wvT", [D, 192], FP16, kind="ExternalInput").ap()
    woT = nc.dram_tensor("woT", [192, D], FP16, kind="ExternalInput").ap()
    cc = nc.dram_tensor("cc", [128, NK], FP16, kind="ExternalInput").ap()
    ss = nc.dram_tensor("ss", [128, NK], FP16, kind="ExternalInput").ap()
    onescol = nc.dram_tensor("onescol", [128, 57], FP16, kind="ExternalInput").ap()
    identd = nc.dram_tensor("identd", [128, 128], FP16, kind="ExternalInput").ap()
    out = nc.dram_tensor("out", [NQ, D], FP16, kind="ExternalOutput").ap()

    const = ctx.enter_context(tc.tile_pool(name="const", bufs=1))
    xpool = ctx.enter_context(tc.tile_pool(name="xp", bufs=2))
    tcsp = ctx.enter_context(tc.tile_pool(name="tcs", bufs=2))
    tsswp = ctx.enter_context(tc.tile_pool(name="tssw", bufs=1))
    qsump = ctx.enter_context(tc.tile_pool(name="qsum", bufs=2))
    qcp = ctx.enter_context(tc.tile_pool(name="qc", bufs=2))
    ptp = ctx.enter_context(tc.tile_pool(name="pt", bufs=6))
    ptp18 = ctx.enter_context(tc.tile_pool(name="pt18", bufs=2))
    polyp = ctx.enter_context(tc.tile_pool(name="poly", bufs=1))
    unqp = ctx.enter_context(tc.tile_pool(name="unq", bufs=2))
    recp = ctx.enter_context(tc.tile_pool(name="rec", bufs=2))
    osbp = ctx.enter_context(tc.tile_pool(name="osb", bufs=5))
    abp = ctx.enter_context(tc.tile_pool(name="ab", bufs=2))

    # PSUM: sg 2x3 banks + shared misc 2x1 banks = 8
    ps_sg = ctx.enter_context(tc.tile_pool(name="ps_sg", bufs=2, space="PSUM"))
    ps_ms = ctx.enter_context(tc.tile_pool(name="ps_ms", bufs=2, space="PSUM"))

    # ---- constants -------------------------------------------------------
    wq_sb = const.tile([128, 6, 384], FP16)
    wqr = wqkT.rearrange("(c p) m -> p c m", p=128)
    wv_sb = const.tile([128, 6, 192], FP16)
    wvr = wvT.rearrange("(c p) m -> p c m", p=128)
    cc_sb = const.tile([128, NK], FP16)
    ss_sb = const.tile([128, NK], FP16)
    wo01 = const.tile([128, D], FP16)
    wo2 = const.tile([64, D], FP16)
    ident = const.tile([128, 128], FP16)

    # fp8 q/k tiles: q [64, 2, NQ] (half0 = fp8(8q), half1 = residual),
    # k [64, 2336] (cols to 2336 for the chunk-18 read); pad columns are
    # zeroed by the rope writes themselves
    q8t = [const.tile([64, 2, NQ], FP8, name=f"q8_{h}") for h in range(HPC)]
    k8t = [const.tile([64, KW], FP8, name=f"k8_{h}") for h in range(HPC)]

    v_sb = const.tile([128, NCH * HPC * VE], FP16)
    v_sb4 = v_sb.rearrange("p (c h e) -> p c h e", c=NCH, h=HPC)
    # chunk-18 v rows replicated at partition base 32h per head, so the
    # heads-batched pt18 tile can feed PV directly
    v18 = const.tile([66, HPC, VE], FP16)
    # preload the Exp activation table while the first DMAs are in flight
    warm = const.tile([1, 8], FP32)
    nc.gpsimd.memset(warm, 0.0)

    # rope targets: mi -> [(kind, head) for g in 0..1]; kind q/k
    rope_tgt = {
        0: [("q", 0), ("q", 1)],
        1: [("q", 2), ("k", 0)],
        2: [("k", 1), ("k", 2)],
    }
    pt_tiles = {}
    pt18_tiles = {}
    ab_tiles = {}

    def scores18(pi):
        # key chunk 18 has only 2 real keys; batch all 3 heads' scores at
        # partition bases 0/32/64 and exp them in ONE activation. Rows
        # 2304..2335 include 30 zero-pad keys so the exp input is defined.
        # fp8 non-DR matmuls on half0 only (no ones-row, no residual).
        off, w = PAN2[pi]
        sg18 = ps_sg.tile([128, 512], FP32, tag="sg", name=f"sg18_{pi}")
        for h in range(HPC):
            nc.tensor.matmul(
                sg18[32 * h:32 * h + 32, :w],
                lhsT=k8t[h][0:64, 2304:2336],
                rhs=q8t[h][0:64, 0, off:off + w],
                start=True,
                stop=True,
            )
        pt18 = ptp18.tile([66, 512], FP16, tag="pt18", name=f"pt18_{pi}")
        pt18_tiles[pi] = pt18
        nc.scalar.activation(pt18[:, :w], sg18[0:66, :w], AF.Exp,
                             scale=0.125 / 64.0)

    def get_pt(pi, h):
        if (pi, h) not in pt_tiles:
            pt_tiles[(pi, h)] = ptp.tile(
                [128, NCH - 1, 512], FP16, tag="pt", name=f"pt{pi}_{h}"
            )
        return pt_tiles[(pi, h)]

    def score_mms(pi, h, c0, cnt):
        off, w = PAN2[pi]
        sg = ps_sg.tile([128, 1536], FP32, tag="sg", name=f"sg{pi}_{h}_{c0}")
        sg3 = sg.rearrange("p (c q) -> p c q", c=3)
        kbc = k8t[h].unsqueeze(1).broadcast_to([64, 2, KW])
        for j in range(cnt):
            c = c0 + j
            nc.tensor.matmul(
                sg3[:, j, :w],
                lhsT=kbc[:, :, c * 128:(c + 1) * 128],
                rhs=q8t[h][:, :, off:off + w],
                start=True,
                stop=True,
                perf_mode=DR,
            )
        return sg3

    def scores_group(pi, h, c0, cnt):
        off, w = PAN2[pi]
        pt = get_pt(pi, h)
        sg3 = score_mms(pi, h, c0, cnt)
        nc.scalar.activation(
            pt[:, c0:c0 + cnt, :w], sg3[:, 0:cnt, :w], AF.Exp,
            scale=0.125 / 64.0,
        )

    def poly_group(pi, h, c0, cnt):
        # exp offloaded: (1 + t + t^2/2)^16 with t = s/128. The single
        # PSUM read (DVE tensor_scalar) makes c = sg*2^-13 = 1+t; Pool
        # runs the fp32 stages; DVE finishes with two fp16 squarings.
        off, w = PAN2[pi]
        pt = get_pt(pi, h)
        sg3 = score_mms(pi, h, c0, cnt)
        zf = polyp.tile([128, 3, 512], FP32, tag="zf")
        zf2 = polyp.tile([128, 3, 512], FP32, tag="zf2")
        zh = polyp.tile([128, 3, 512], FP16, tag="zh")
        zh2 = polyp.tile([128, 3, 512], FP16, tag="zh2")
        nc.vector.tensor_scalar(
            out=zf[:, 0:cnt, :w], in0=sg3[:, 0:cnt, :w],
            scalar1=float(2.0 ** -13), scalar2=0.0, op0=OP.mult, op1=OP.add,
        )
        nc.gpsimd.tensor_tensor(
            out=zf2[:, 0:cnt, :w], in0=zf[:, 0:cnt, :w],
            in1=zf[:, 0:cnt, :w], op=OP.mult,
        )
        nc.gpsimd.tensor_scalar(
            out=zf[:, 0:cnt, :w], in0=zf2[:, 0:cnt, :w],
            scalar1=0.5, scalar2=0.5, op0=OP.mult, op1=OP.add,
        )
        nc.gpsimd.tensor_tensor(
            out=zf2[:, 0:cnt, :w], in0=zf[:, 0:cnt, :w],
            in1=zf[:, 0:cnt, :w], op=OP.mult,
        )
        nc.gpsimd.tensor_tensor(
            out=zh[:, 0:cnt, :w], in0=zf2[:, 0:cnt, :w],
            in1=zf2[:, 0:cnt, :w], op=OP.mult,
        )
        nc.gpsimd.tensor_tensor(
            out=zh2[:, 0:cnt, :w], in0=zh[:, 0:cnt, :w],
            in1=zh[:, 0:cnt, :w], op=OP.mult,
        )
        nc.gpsimd.tensor_tensor(
            out=pt[:, c0:c0 + cnt, :w], in0=zh2[:, 0:cnt, :w],
            in1=zh2[:, 0:cnt, :w], op=OP.mult,
        )

    def scores_exp(pi, h, groups=None):
        gl = list(enumerate(groups if groups is not None else GROUPS))
        gl.sort(key=lambda t: (pi, h, t[0]) not in POLY)
        for gi, (c0, cnt) in gl:
            if (pi, h, gi) in POLY:
                poly_group(pi, h, c0, cnt)
            else:
                scores_group(pi, h, c0, cnt)

    def get_ab(pi):
        if pi not in ab_tiles:
            ab01 = abp.tile([128, 512], FP16, tag="ab01", name=f"ab01_{pi}")
            ab2 = abp.tile([64, 512], FP16, tag="ab2", name=f"ab2_{pi}")
            ab_tiles[pi] = (ab01, ab2)
        return ab_tiles[pi]

    def pv_qsub(pi, h, q0, qw, tp_pool="ms", po_pool="ms"):
        off, w = PAN2[pi]
        pt = pt_tiles[(pi, h)]
        ab01, ab2 = get_ab(pi)
        abt, ab_base = [(ab01, 0), (ab01, 64), (ab2, 0)][h]
        po = (ps_ms if po_pool == "ms" else ps_sg).tile(
            [128, 512], FP32, tag=po_pool, name=f"po{pi}_{h}_{q0}"
        )
        pt18 = pt18_tiles[pi]
        for c in range(NCH - 1):
            nc.tensor.matmul(
                po[:qw, 0:VE],
                lhsT=pt[:, c, q0:q0 + qw],
                rhs=v_sb4[:, c, h, :],
                start=(c == 0),
                stop=False,
            )
        nc.tensor.matmul(
            po[:qw, 0:VE],
            lhsT=pt18[32 * h:32 * h + 2, q0:q0 + qw],
            rhs=v18[32 * h:32 * h + 2, h, :],
            start=False,
            stop=True,
        )
        # normalize in place: unq = po[:, 0:64] * (1 / den)
        rec = recp.tile([128, 1], FP32, tag="rec")
        with nc.allow_low_precision(reason="softmax denominators are ~2e3"):
            nc.vector.reciprocal(rec[:qw, :], po[:qw, 64:65])
        unq = unqp.tile([128, 64], FP16, tag="unq")
        nc.vector.tensor_scalar_mul(unq[:qw, :], po[:qw, 0:64], rec[:qw, :])
        tp = (ps_ms if tp_pool == "ms" else ps_sg).tile(
            [128, 1024], FP16, tag=tp_pool, name=f"tp{pi}_{h}_{q0}"
        )
        nc.tensor.transpose(tp[0:64, 0:qw], unq[:qw, 0:64], ident[0:qw, 0:qw])
        nc.vector.tensor_copy(
            abt[ab_base:ab_base + 64, q0:q0 + qw], tp[0:64, 0:qw]
        )

    def pv_head(pi, h, pool="ms"):
        off, w = PAN2[pi]
        q0 = 0
        while q0 < w:
            qw = min(128, w - q0)
            pv_qsub(pi, h, q0, qw, tp_pool=pool, po_pool=pool)
            q0 += qw
        pt_tiles.pop((pi, h))

    def outproj(pi, q0, qw, pool="ms", act_osb=False):
        off, _ = PAN2[pi]
        ab01, ab2 = ab_tiles[pi]
        t0 = q0
        while t0 < q0 + qw:
            tw = min(128, q0 + qw - t0)
            for half in range(2):
                op_ps = (ps_ms if pool == "ms" else ps_sg).tile(
                    [128, 512], FP32, tag=pool, name=f"op{pi}_{t0}_{half}"
                )
                nc.tensor.matmul(
                    op_ps[:tw, 0:384],
                    lhsT=ab01[:, t0:t0 + tw],
                    rhs=wo01[:, half * 384:half * 384 + 384],
                    start=True,
                    stop=False,
                )
                nc.tensor.matmul(
                    op_ps[:tw, 0:384],
                    lhsT=ab2[0:64, t0:t0 + tw],
                    rhs=wo2[:, half * 384:half * 384 + 384],
                    start=False,
                    stop=True,
                )
                osb = osbp.tile([128, 384], FP16, tag="osb")
                if act_osb:
                    nc.scalar.copy(osb[:tw, :], op_ps[:tw, 0:384])
                else:
                    nc.vector.tensor_copy(osb[:tw, :], op_ps[:tw, 0:384])
                nc.sync.dma_start(
                    out=out[off + t0:off + t0 + tw,
                            half * 384:half * 384 + 384],
                    in_=osb[:tw, :],
                )
            t0 += tw

    # ---- phase 1: QK projection + rope; V in [tok, e]; early scores ------
    xTr = xT.rearrange("(c p) n -> p c n", p=128)
    for pi, (off, w) in enumerate(PAN1):
        xp = xpool.tile([128, 6, 512], FP16, tag="xp")
        if pi == 0:
            # startup DMAs spread across idle engine queues so the x panel,
            # weights and rope tables land in parallel
            nc.sync.dma_start(out=xp[:, :, :w], in_=xTr[:, :, off:off + w])
            nc.scalar.dma_start(out=wq_sb, in_=wqr)
            nc.scalar.dma_start(out=cc_sb, in_=cc)
            nc.scalar.dma_start(out=ss_sb, in_=ss)
            nc.scalar.activation(warm, warm, AF.Exp, scale=1.0)
            nc.sync.dma_start(out=wv_sb, in_=wvr)
            # PE p-state warm-up: dummy full-shape matmuls keep the PE
            # continuously busy from ~t=1us so the real projections run
            # at full clock; outputs are never read
            pew = const.tile([128, 512], FP16, name="pew")
            nc.gpsimd.memset(pew, 0.0)
            for wi in range(6):
                wps = ps_ms.tile([128, 512], FP32, tag="ms", name=f"pew{wi}")
                nc.tensor.matmul(
                    wps[:, :], lhsT=pew[:, 0:128], rhs=pew[:, :],
                    start=True, stop=True,
                )
        else:
            nc.sync.dma_start(out=xp[:, :, :w], in_=xTr[:, :, off:off + w])
        if pi == 1:
            nc.sync.dma_start(
                out=v_sb4[:, :, :, 64:65],
                in_=onescol.rearrange("p (c h) -> p c h", c=NCH),
            )
            nc.sync.dma_start(out=ident, in_=identd)
            nc.sync.dma_start(out=wo01, in_=woT[0:128, :])
            nc.sync.dma_start(out=wo2, in_=woT[128:192, :])
        # qk projection chunks + rope; mults on DVE, rest on Pool
        for mi in range(3):
            qp = ps_ms.tile([128, 512], FP32, tag="ms", name=f"qk{pi}_{mi}")
            for kc in range(6):
                nc.tensor.matmul(
                    qp[:, :w],
                    lhsT=wq_sb[:, kc, mi * 128:(mi + 1) * 128],
                    rhs=xp[:, kc, :w],
                    start=(kc == 0),
                    stop=(kc == 5),
                )
            qpc = qcp.tile([128, 512], FP16, tag="qpc")
            nc.vector.tensor_copy(qpc[:, :w], qp[:, :w])
            tcs = tcsp.tile([128, 512], FP16, tag="tcs")
            nc.vector.tensor_tensor(
                out=tcs[:, :w], in0=qpc[:, :w], in1=cc_sb[:, off:off + w],
                op=OP.mult,
            )
            tss = tcsp.tile([128, 512], FP16, tag="tss")
            nc.vector.tensor_tensor(
                out=tss[:, :w], in0=qpc[:, :w], in1=ss_sb[:, off:off + w],
                op=OP.mult,
            )
            # DVE swap-copies tss with the sign folded in (fp16 4x mode),
            # then one Pool add per 64-block target
            tssw = tsswp.tile([128, 512], FP16, tag="tssw")
            for g in range(2):
                r = slice(g * 64, g * 64 + 32)
                i = slice(g * 64 + 32, g * 64 + 64)
                nc.vector.tensor_scalar_mul(tssw[r, :w], tss[i, :w], -1.0)
                nc.vector.tensor_copy(tssw[i, :w], tss[r, :w])
            for g in range(2):
                kind, h = rope_tgt[mi][g]
                if kind == "k":
                    kw = min(w, KW - off)
                    if kw <= 0:
                        continue
                    nc.gpsimd.tensor_tensor(
                        out=k8t[h][0:64, off:off + kw],
                        in0=tcs[g * 64:g * 64 + 64, :kw],
                        in1=tssw[g * 64:g * 64 + 64, :kw],
                        op=OP.add,
                    )
                else:
                    qw = min(w, NQ - off)
                    if qw <= 0:
                        continue
                    qsum = qsump.tile([64, 512], FP16, tag="qsum")
                    nc.gpsimd.tensor_tensor(
                        out=qsum[:, :qw],
                        in0=tcs[g * 64:g * 64 + 64, :qw],
                        in1=tssw[g * 64:g * 64 + 64, :qw],
                        op=OP.add,
                    )
                    nc.gpsimd.tensor_copy(
                        q8t[h][0:64, 0, off:off + qw], qsum[:, :qw]
                    )
                    nc.gpsimd.tensor_tensor(
                        out=q8t[h][0:64, 1, off:off + qw],
                        in0=qsum[:, :qw],
                        in1=q8t[h][0:64, 0, off:off + qw],
                        op=OP.subtract,
                    )
        # early scores for query-panels 0/1 on this panel's key chunks
        for qp_, g in EARLY[pi]:
            if g == 18:
                scores18(qp_)
            else:
                for h in range(HPC):
                    scores_group(qp_, h, *GROUPS[g])
        # v projection for this panel's key chunks, [tok, e] orientation
        for t0 in range(0, w, 128):
            ci = (off + t0) // 128
            vps = ps_ms.tile([128, 512], FP32, tag="ms", name=f"vp{ci}")
            for kc in range(6):
                nc.tensor.matmul(
                    vps[:, 0:192],
                    lhsT=xp[:, kc, t0:t0 + 128],
                    rhs=wv_sb[:, kc, :],
                    start=(kc == 0),
                    stop=(kc == 5),
                )
            nc.vector.tensor_copy(
                v_sb4[:, ci, :, 0:64],
                vps[:, 0:192].rearrange("p (h e) -> p h e", h=HPC),
            )
            if ci == NCH - 1:
                for h in range(HPC):
                    nc.gpsimd.tensor_copy(
                        v18[32 * h:32 * h + 2, h, :], v_sb4[0:2, ci, h, :]
                    )

    if phases == 1:
        return

    # ---- phase 2: attention ---------------------------------------------
    # emission order keeps Act (exp) saturated: tail work of panel p-1
    # threads between panel p's score blocks; panels 0/1 already scored
    npan = len(PAN2)
    pv_head(0, 0)
    pv_head(0, 1)
    pv_head(0, 2)
    outproj(0, 0, PAN2[0][1])
    pv_head(1, 0)
    pv_head(1, 1)
    for pi in range(2, npan):
        scores_exp(pi, 0)
        scores18(pi)
        pv_head(pi - 1, 2)
        if pi < npan - 1:
            outproj(pi - 1, 0, PAN2[pi - 1][1])
        scores_exp(pi, 1)
        pv_head(pi, 0)
        scores_exp(pi, 2)
        # the last panel's h1 chains ride the sg pool (free after the
        # final exps), keeping ms clear for outproj
        pv_head(pi, 1, pool="sg" if pi == npan - 1 else "ms")
        if pi == npan - 1:
            outproj(pi - 1, 0, PAN2[pi - 1][1])
    # last panel tail: h2's chains interleave with per-qsub projections,
    # transposes ride the now-idle sg pool for extra pipeline depth
    offl, wl = PAN2[npan - 1]
    q0 = 0
    while q0 < wl:
        qw = min(128, wl - q0)
        pv_qsub(npan - 1, 2, q0, qw, tp_pool="sg", po_pool="sg")
        outproj(npan - 1, q0, qw, act_osb=True)
        q0 += qw
    pt_tiles.pop((npan - 1, 2))


_NC_CACHE = {}


def build_nc(trace_sim=False, phases=3):
    key = (bool(trace_sim), phases)
    if key in _NC_CACHE:
        return _NC_CACHE[key]
    from contextlib import ExitStack

    nc = bass.Bass("TRN2", target_bir_lowering=False, debug=False, num_devices=8)
    with tile.TileContext(nc, trace_sim=trace_sim) as tc:
        with ExitStack() as ctx:
            _emit(nc, tc, ctx, phases=phases)
    split_excess_waits(nc)
    _NC_CACHE[key] = nc
    return nc


def host_prep(x, pos0, pos1, wq, wk, wv, wo, core):
    """Per-core DRAM inputs. core -> batch b=core//4, heads 3*(core%4)+[0..2]."""
    import ml_dtypes
    fp16 = np.float16
    b = core // 4
    h0 = 3 * (core % 4)
    hs = [h0, h0 + 1, h0 + 2]

    xT = np.zeros((D, NK), np.float32)
    xT[:, :N] = x[b].T

    def perm_rows(w_h):  # evens then odds of the head dim
        return np.concatenate([w_h[0::2], w_h[1::2]], axis=0)

    wq_rows = np.concatenate([perm_rows(wq[h * E:(h + 1) * E]) for h in hs], 0)
    wk_rows = np.concatenate([perm_rows(wk[h * E:(h + 1) * E]) for h in hs], 0)
    wqkT = np.ascontiguousarray(np.concatenate([wq_rows, wk_rows], 0).T)
    wv_rows = np.concatenate([wv[h * E:(h + 1) * E] for h in hs], 0)
    wvT = np.ascontiguousarray(wv_rows.T)

    wo_cols = np.concatenate([wo[:, h * E:(h + 1) * E] for h in hs], 1)
    woT = np.ascontiguousarray(wo_cols.T)

    theta = 1.0 / (BASE ** (np.arange(KMAX, dtype=np.float32) / KMAX))
    i1, i2 = np.meshgrid(np.arange(N1), np.arange(N2), indexing="ij")
    ang0 = pos0[b][i1.ravel()][:, None] * theta[None, :]
    ang1 = pos1[b][i2.ravel()][:, None] * theta[None, :]
    ang = np.concatenate([ang0, ang1], 1).astype(np.float32)  # [N-1, 32]
    cos = np.ones((32, NK), np.float32)   # col 0 (CLS) and pad cols: identity
    sin = np.zeros((32, NK), np.float32)
    cos[:, 1:N] = np.cos(ang).T
    sin[:, 1:N] = np.sin(ang).T
    # x8 fp8 scaling folded into the rope tables
    cc = np.ascontiguousarray(np.tile(cos * 8.0, (4, 1))).astype(fp16)
    ss = np.ascontiguousarray(np.tile(sin * 8.0, (4, 1))).astype(fp16)
    onescol = np.ones((128, NCH, HPC), np.float32)
    onescol[1:, NCH - 1, :] = 0.0  # pad keys contribute nothing
    identd = np.eye(128, dtype=np.float32)
    return {
        "xT": xT.astype(fp16),
        "wqkT": wqkT.astype(fp16),
        "wvT": wvT.astype(fp16),
        "woT": woT.astype(fp16),
        "cc": cc, "ss": ss,
        "onescol": np.ascontiguousarray(
            onescol.reshape(128, NCH * HPC)).astype(fp16),
        "identd": identd.astype(fp16),
    }


def kernel(x, pos0, pos1, wq, wk, wv, wo):
    x = np.asarray(x, np.float32)
    pos0 = np.asarray(pos0, np.float32)
    pos1 = np.asarray(pos1, np.float32)
    wq = np.asarray(wq, np.float32)
    wk = np.asarray(wk, np.float32)
    wv = np.asarray(wv, np.float32)
    wo = np.asarray(wo, np.float32)

    in_maps = [
        host_prep(x, pos0, pos1, wq, wk, wv, wo, core) for core in range(8)
    ]
    nc = build_nc()
    res = run_bass_kernel_spmd(nc, in_maps, core_ids=list(range(8)))
    out = np.zeros((B, N, D), np.float32)
    for core in range(8):
        out[core // 4] += np.asarray(res.results[core]["out"][:N], np.float32)
    return out
